# revision 21
# baseline (speedup 1.0000x reference)
"""CLAM-SB MIL forward on 8 Trainium2 NeuronCores (Bass/Tile).

Data-parallel over the bag dimension: core b handles bag b.

Host prep: X[b] is cast to fp16 and pre-transposed to Xt [1024, 16384]
(d-major), and the per-bag classifier weights are packed into one rhs
matrix Wa = [W1 | Wd0 | Wd1 | Wd2 | Wc] (1024 x 132, fp16), where Wd* are
the logit-difference columns of the label-selected instance classifiers.

Device, single pass over Xt (one matmul family does everything):
  for each 128-row tile t: psum[t] = Xt_chunk(t)^T-stationary @ Wa
    -> [128 rows(n) x 132]: cols 0..127 = h (pre-tanh), 128..130 = instance
       logit diffs, 131 = per-instance bag score c_n = x_n . Wc
  f = w2 . tanh(h + b1) via DVE dot -> u = exp(f) grid [128, 128]
  (col t = row-tile t), w = u * (mask>0); logits+c copied to Lgrid.
No PE transposes, no z matmuls, no tail gather: bag_pred = sum(w*c)/L,
and the instance losses are softplus over the FULL logit grids masked by
the top/bottom-64 selections (thresholds from per-partition top-8
candidates + max8/match_replace rounds, as before).
Host combines the per-core scalars into the reference's [10] output.

build_kernel(rep=K) wraps the whole per-core body in a tc.For_i hardware
loop executing it K times back-to-back in one NEFF - used only for timing
(the slope of wall time in K isolates per-body device time from RPC
dispatch costs). The graded kernel() path uses rep=1.
"""
import numpy as np

import concourse.bacc as bacc
import concourse.bass as bass
import concourse.mybir as mybir
import concourse.tile as tile
from concourse import bass_utils

f32 = mybir.dt.float32
f16 = mybir.dt.float16
u32 = mybir.dt.uint32
AluOp = mybir.AluOpType
AFT = mybir.ActivationFunctionType
AX = mybir.AxisListType

N, D, A = 16384, 1024, 128
NT = N // 128           # 128 row-tiles
NG = NT // 4            # 32 groups of 4 tiles
SBG = 2                 # groups per DMA superblock (1024 rows)
NSB = NG // SBG         # 16 superblocks
KW = 132                # rhs width: 128 h + 3 logit diffs + 1 bag score
NEG = -1.0e30


def build_kernel(stage=99, rep=1, with_b1=False):
    nc = bacc.Bacc("TRN2", target_bir_lowering=False, debug=False, num_devices=8)
    # Xt is stored in stream order: [sb, p, c, nn] so each superblock DMA
    # reads one contiguous 16 KB block per partition (sequential HBM access).
    Xt = nc.dram_tensor("Xt", [NSB, 128, 8, SBG * 512], f16,
                        kind="ExternalInput").ap()
    Wa = nc.dram_tensor("Wa", [D, KW], f16, kind="ExternalInput").ap()
    w2r = nc.dram_tensor("w2r", [128, 8, 128], f32, kind="ExternalInput").ap()
    b1rep = (nc.dram_tensor("b1rep", [128, 2, 128], f32, kind="ExternalInput").ap()
             if with_b1 else None)
    maskg = nc.dram_tensor("maskg", [128, 128], f32, kind="ExternalInput").ap()
    cbr = nc.dram_tensor("cbr", [128, 4], f32, kind="ExternalInput").ap()
    out_vec = nc.dram_tensor("out_vec", [1, 8], f32, kind="ExternalOutput").ap()
    out_cnt = nc.dram_tensor("out_cnt", [2, 2], f32, kind="ExternalOutput").ap()

    with tile.TileContext(nc) as tc:
        consts = tc.alloc_tile_pool(name="consts", bufs=1)
        Wasb = consts.tile([128, 8, KW], f16)
        nc.sync.dma_start(Wasb[:], Wa.rearrange("(c p) k -> p c k", p=128))
        w2sb = consts.tile([128, 8, 128], f32)
        nc.sync.dma_start(w2sb[:], w2r[:])
        if with_b1:
            b1sb = consts.tile([128, 2, 128], f32)
            nc.sync.dma_start(b1sb[:], b1rep[:])
        masksb = consts.tile([128, 128], f32)
        nc.sync.dma_start(masksb[:], maskg[:])
        mask01 = consts.tile([128, 128], f32)
        nc.vector.tensor_scalar(mask01[:], masksb[:], 0.0, None, op0=AluOp.is_gt)
        ones1 = consts.tile([1, 128], f32)
        nc.vector.memset(ones1[:], 1.0)
        cbsb = consts.tile([128, 4], f32)
        nc.sync.dma_start(cbsb[:], cbr[:])
        onesc = consts.tile([128, 4], f32)
        nc.vector.memset(onesc[:], 1.0)

        # persistent grids: [p, t] = row n = 128*t + p
        u_grid = consts.tile([128, 128], f32)     # exp(f)
        w_grid = consts.tile([128, 128], f32)     # u * mask01
        Lgrid = consts.tile([128, 128, 4], f32)   # logit diffs 0..2, col 3 = c_n

        def emit_body():
            xp = tc.alloc_tile_pool(name="xp", bufs=2)
            thp = tc.alloc_tile_pool(name="thp", bufs=2)
            ps_h = tc.alloc_tile_pool(name="ps_h", bufs=3, space="PSUM")

            for sb in range(NSB):
                xsb = xp.tile([128, 8, SBG * 512], f16, name=f"x{sb}", tag="x",
                              bufs=4)
                nc.sync.dma_start(xsb[:, 0:4], Xt[sb, :, 0:4])
                nc.sync.dma_start(xsb[:, 4:8], Xt[sb, :, 4:8])
                if stage == 0:
                    # DMA-only ablation: touch the tile so it isn't dead
                    probe = thp.tile([128, 1], f16, name=f"pr{sb}", tag="pr")
                    nc.vector.tensor_copy(probe[:], xsb[:, 0, 0:1])
                    continue
                phs = []
                for gi in range(SBG):
                    g = sb * SBG + gi
                    for h2 in range(2):
                        ph = ps_h.tile([128, 2, KW], f32, name=f"ph{g}_{h2}",
                                       tag=f"ph{h2}", bufs=4)
                        phs.append(ph)
                        for tt in range(2):
                            off = gi * 512 + (2 * h2 + tt) * 128
                            for c in range(8):
                                nc.tensor.matmul(ph[:, tt, :],
                                                 xsb[:, c, off:off + 128],
                                                 Wasb[:, c, :],
                                                 start=(c == 0), stop=(c == 7))
                        if with_b1:
                            nc.vector.tensor_tensor(
                                ph[:, :, 0:128], ph[:, :, 0:128],
                                b1sb[:], op=AluOp.add)
                if stage == 1:
                    probe = thp.tile([128, 1], f32, name=f"pg{sb}", tag="pr1")
                    nc.vector.tensor_copy(probe[:], phs[0][:, 0, 0:1])
                    continue
                # tanh straight from PSUM, then f = sum_a th * w2
                th = thp.tile([128, 8, 128], f32, name=f"th{sb}", tag="th")
                for q in range(4):
                    nc.scalar.activation(th[:, 2 * q:2 * q + 2, :],
                                         phs[q][:, :, 0:128], AFT.Tanh,
                                         bias=0.0, scale=1.0)
                scr = thp.tile([128, 8, 128], f32, name=f"sc{sb}", tag="sc")
                nc.vector.tensor_tensor(scr[:], th[:], w2sb[:], op=AluOp.mult)
                fcol = thp.tile([128, 8], f32, name=f"f{sb}", tag="f")
                nc.vector.tensor_reduce(
                    fcol[:].rearrange("p (f o) -> p f o", o=1),
                    scr[:], axis=AX.X, op=AluOp.add)
                nc.scalar.activation(u_grid[:, 8 * sb:8 * sb + 8], fcol[:],
                                     AFT.Exp, bias=0.0, scale=1.0)
                nc.vector.tensor_tensor(w_grid[:, 8 * sb:8 * sb + 8],
                                        u_grid[:, 8 * sb:8 * sb + 8],
                                        mask01[:, 8 * sb:8 * sb + 8],
                                        op=AluOp.mult)
                # logit diffs (+cb) and bag scores into Lgrid
                for q in range(4):
                    dst = Lgrid[:, 8 * sb + 2 * q:8 * sb + 2 * q + 2, :]
                    if q % 2 == 0:
                        nc.vector.tensor_copy(dst, phs[q][:, :, 128:132])
                    else:
                        nc.scalar.copy(dst, phs[q][:, :, 128:132])

            ps_h.release()

            if stage <= 2:
                tailp = tc.alloc_tile_pool(name="tailp", bufs=1)
                outt = tailp.tile([1, 8], f32)
                nc.vector.memset(outt[:], 0.0)
                nc.sync.dma_start(out_vec[:], outt[:])
                cnts = tailp.tile([2, 2], f32)
                nc.vector.memset(cnts[:], 0.0)
                nc.sync.dma_start(out_cnt[:], cnts[:])
                tailp.release()
                thp.release()
                xp.release()
                return

            # ---------- tail ----------
            tailp = tc.alloc_tile_pool(name="tailp", bufs=1)
            ps_t = tc.alloc_tile_pool(name="ps_t", bufs=1, space="PSUM")

            # L = sum(w_grid); bag dot = sum(w * c)
            S4 = tailp.tile([128, 4], f32)
            nc.vector.tensor_reduce(S4[:, 0:1], w_grid[:], axis=AX.X, op=AluOp.add)
            pL = ps_t.tile([1, 4], f32)
            nc.tensor.matmul(pL[:], S4[:, 0:1], onesc[:], start=True, stop=True)
            recipL = tailp.tile([1, 1], f32)
            nc.vector.reciprocal(recipL[:], pL[:, 0:1])
            cw = tailp.tile([128, 128], f32)
            nc.vector.tensor_tensor(cw[:], w_grid[:], Lgrid[:, :, 3], op=AluOp.mult)

            # softplus grids depend only on Lgrid: run on ACT while the DVE
            # does the rank rounds below
            sps = []
            for k in range(3):
                ee = tailp.tile([128, 128], f32, name=f"ee{k}")
                nc.scalar.activation(ee[:], Lgrid[:, :, k], AFT.Exp,
                                     bias=cbsb[:, k:k + 1], scale=1.0)
                sp = tailp.tile([128, 128], f32, name=f"sp{k}")
                nc.scalar.activation(sp[:], ee[:], AFT.Ln, bias=1.0, scale=1.0)
                sps.append(sp)

            # candidates: top-8 per partition of u (and of -u)
            v8 = tailp.tile([128, 8], f32)
            nc.vector.max(v8[:], u_grid[:])
            uneg = tailp.tile([128, 128], f32)
            nc.vector.tensor_scalar(uneg[:], u_grid[:], -1.0, None, op0=AluOp.mult)
            v8b = tailp.tile([128, 8], f32)
            nc.vector.max(v8b[:], uneg[:])

            # consolidate candidate values to [2, 1024] rows (p-major: col = 8p+c)
            cand2 = tailp.tile([2, 1024], f32)
            nc.sync.dma_start(cand2[0:1, :], v8[:])
            nc.sync.dma_start(cand2[1:2, :], v8b[:])
            candB0 = tailp.tile([1, 1024], f32)
            nc.sync.dma_start(candB0[:], v8b[:])

            # threshold: 8 rounds of max8 + match_replace -> 64th; one more
            # max8 -> 65th; thr = midpoint
            work = tailp.tile([2, 1024], f32)
            nc.vector.tensor_copy(work[:], cand2[:])
            m8 = tailp.tile([2, 8], f32)
            v64 = tailp.tile([2, 1], f32)
            for r in range(8):
                nc.vector.max(m8[:], work[:])
                if r == 7:
                    nc.vector.tensor_copy(v64[:], m8[:, 7:8])
                nc.vector.match_replace(work[:], m8[:], work[:], NEG)
            m8b = tailp.tile([2, 8], f32)
            nc.vector.max(m8b[:], work[:])
            thr2 = tailp.tile([2, 1], f32)
            nc.vector.tensor_scalar(thr2[:], v64[:], m8b[:, 0:1], 0.5,
                                    op0=AluOp.add, op1=AluOp.mult)

            # candidate-space selections -> counts + 8th-slot guard (out_cnt)
            selTB = tailp.tile([2, 1024], f32)
            nc.vector.tensor_scalar(selTB[:], cand2[:], thr2[:, :1], None,
                                    op0=AluOp.is_gt)
            cnt2 = tailp.tile([2, 2], f32)
            nc.vector.tensor_reduce(cnt2[:, 0:1], selTB[:], axis=AX.X, op=AluOp.add)
            nc.vector.tensor_reduce(
                cnt2[:, 1:2].rearrange("q (a o) -> q a o", a=1),
                selTB[:].rearrange("q (p j) -> q p j", p=128)[:, :, 7:8],
                axis=AX.XY, op=AluOp.add)
            nc.sync.dma_start(out_cnt[:], cnt2[:])

            # broadcast thresholds to all partitions: thrps = ones1^T @ [thrT thrB]
            thrrow = tailp.tile([1, 2], f32)
            nc.sync.dma_start(thrrow[:], thr2[:])
            thrps = ps_t.tile([128, 2], f32)
            nc.tensor.matmul(thrps[:], ones1[:], thrrow[:], start=True, stop=True)
            thrsb = tailp.tile([128, 2], f32)
            nc.vector.tensor_copy(thrsb[:], thrps[:])

            # grid-space selections
            selgT = tailp.tile([128, 128], f32)
            nc.vector.tensor_scalar(selgT[:], u_grid[:], thrsb[:, 0:1], None,
                                    op0=AluOp.is_gt)
            selgB = tailp.tile([128, 128], f32)
            nc.vector.tensor_scalar(selgB[:], uneg[:], thrsb[:, 1:2], None,
                                    op0=AluOp.is_gt)

            # masked sums over the precomputed softplus grids
            # slot 1: top sel, diff col 0; slot 2: bottom sel, col 1;
            # slot 3: top sel, col 2
            for slot, (k, selg) in enumerate(
                    [(0, selgT), (1, selgB), (2, selgT)], start=1):
                ws = tailp.tile([128, 128], f32, name=f"ws{k}{slot}")
                nc.vector.tensor_tensor(ws[:], sps[k][:], selg[:], op=AluOp.mult)
                nc.vector.tensor_reduce(S4[:, slot:slot + 1], ws[:], axis=AX.X,
                                        op=AluOp.add)
            # overwrite S4 col 0 with the bag dot now that cw is ready
            nc.vector.tensor_reduce(S4[:, 0:1], cw[:], axis=AX.X, op=AluOp.add)

            # partition sums: pS[m, j] = sum_p S4[p, m]; slot 0 scaled by 1/L
            pS = ps_t.tile([4, 4], f32)
            nc.tensor.matmul(pS[:], S4[:], onesc[:], start=True, stop=True)
            psb = tailp.tile([4, 1], f32)
            nc.scalar.copy(psb[:], pS[:, 0:1])
            nc.vector.tensor_scalar(psb[0:1, :], psb[0:1, :], recipL[:, :1],
                                    None, op0=AluOp.mult)
            nc.sync.dma_start(out_vec[:, 0:4], psb[:])

            ps_t.release()
            tailp.release()
            thp.release()
            xp.release()

        if rep == 1:
            emit_body()
        else:
            with tc.For_i(0, rep, 1, hint_engines=tuple(mybir.ALL_ENGINES)):
                emit_body()

        consts.release()

    nc.compile()
    return nc


_NC_CACHE = {}


def _get_nc(with_b1=False):
    global _NC_CACHE
    if _NC_CACHE is None:
        _NC_CACHE = {}
    if with_b1 not in _NC_CACHE:
        import os
        _NC_CACHE[with_b1] = build_kernel(
            int(os.environ.get("KSTAGE", "99")),
            rep=int(os.environ.get("KREP", "1")), with_b1=with_b1)
    return _NC_CACHE[with_b1]


def make_in_maps(X, mask, labels, W1, b1, w2, b2, Wc, bc, Wi, bi):
    
    X = np.asarray(X, dtype=np.float32)
    mask = np.asarray(mask, dtype=np.float32)
    labels = np.asarray(labels).astype(np.int64)
    W1 = np.asarray(W1, dtype=np.float32)
    b1v = np.asarray(b1, dtype=np.float32).reshape(1, 1, A)
    w2v = np.asarray(w2, dtype=np.float32).reshape(1, 1, A)
    Wc = np.asarray(Wc, dtype=np.float32).reshape(D, 1)
    Wi = np.asarray(Wi, dtype=np.float32)
    bi = np.asarray(bi, dtype=np.float32)
    w2r = np.ascontiguousarray(np.broadcast_to(w2v, (128, 8, A)))
    b1f = np.asarray(b1, dtype=np.float32).reshape(1, A)
    with_b1 = bool(np.any(b1f))
    b1rep = np.ascontiguousarray(
        np.broadcast_to(b1f.reshape(1, 1, A), (128, 2, A)))
    in_maps = []
    for b in range(8):
        lab = int(labels[b])
        Win, Wout = Wi[lab], Wi[1 - lab]
        Wd3 = np.stack([Win[:, 0] - Win[:, 1],
                        Win[:, 1] - Win[:, 0],
                        Wout[:, 1] - Wout[:, 0]], axis=1)  # [1024, 3]
        Wa = np.concatenate([W1, Wd3, Wc], axis=1).astype(np.float16)  # [1024, 132]
        bin_, bout = bi[lab], bi[1 - lab]
        cb = np.array([1.0 + bin_[0] - bin_[1],
                       1.0 + bin_[1] - bin_[0],
                       1.0 + bout[1] - bout[0], 0.0], dtype=np.float32)
        cbrep = np.ascontiguousarray(np.broadcast_to(cb.reshape(1, 4), (128, 4)))
        maskgrid = np.ascontiguousarray(mask[b].reshape(128, 128).T)
        Xtb = X[b].astype(np.float16).T  # [1024(d), 16384(n)] view
        # stream-order layout [sb, p, c, nn]: d = 128*c + p, n = SBG*512*sb + nn
        Xts = np.ascontiguousarray(
            Xtb.reshape(8, 128, 32 // SBG, SBG * 512).transpose(2, 1, 0, 3))
        in_maps.append({
            "Xt": Xts,
            "Wa": np.ascontiguousarray(Wa),
            "w2r": w2r,
            "maskg": maskgrid,
            "cbr": cbrep,
        })
        if with_b1:
            in_maps[-1]["b1rep"] = b1rep
    return in_maps


def assemble(results, labels, bc):
    labels = np.asarray(labels).astype(np.float64)
    bag_pred = np.zeros(8, dtype=np.float64)
    inst = 0.0
    for b in range(8):
        ov = results[b]["out_vec"][0].astype(np.float64)
        bag_pred[b] = ov[0] + float(np.asarray(bc).reshape(-1)[0])
        inst += (ov[1] + ov[2]) / 128.0 + ov[3] / 64.0
    crit = np.mean(np.logaddexp(0.0, bag_pred) - bag_pred * labels)
    out = np.concatenate([bag_pred, [crit], [inst]]).astype(np.float32)
    return out


def kernel(X, mask, labels, W1, b1, w2, b2, Wc, bc, Wi, bi):
    nc = _get_nc(with_b1=bool(np.any(np.asarray(b1))))
    in_maps = make_in_maps(X, mask, labels, W1, b1, w2, b2, Wc, bc, Wi, bi)
    res = bass_utils.run_bass_kernel_spmd(nc, in_maps, core_ids=list(range(8)))
    return assemble(res.results, labels, bc)


# revision 23
# speedup vs baseline: 1.0398x; 1.0398x over previous
"""CLAM-SB MIL forward on 8 Trainium2 NeuronCores (Bass/Tile).

Data-parallel over the bag dimension: core b handles bag b.

Host prep: X[b] is cast to fp16 and pre-transposed to Xt [1024, 16384]
(d-major), and the per-bag classifier weights are packed into one rhs
matrix Wa = [W1 | Wd0 | Wd1 | Wd2 | Wc] (1024 x 132, fp16), where Wd* are
the logit-difference columns of the label-selected instance classifiers.

Device, single pass over Xt (one matmul family does everything):
  for each 128-row tile t: psum[t] = Xt_chunk(t)^T-stationary @ Wa
    -> [128 rows(n) x 132]: cols 0..127 = h (pre-tanh), 128..130 = instance
       logit diffs, 131 = per-instance bag score c_n = x_n . Wc
  f = w2 . tanh(h + b1) via DVE dot -> u = exp(f) grid [128, 128]
  (col t = row-tile t), w = u * (mask>0); logits+c copied to Lgrid.
No PE transposes, no z matmuls, no tail gather: bag_pred = sum(w*c)/L,
and the instance losses are softplus over the FULL logit grids masked by
the top/bottom-64 selections (thresholds from per-partition top-8
candidates + max8/match_replace rounds, as before).
Host combines the per-core scalars into the reference's [10] output.

build_kernel(rep=K) wraps the whole per-core body in a tc.For_i hardware
loop executing it K times back-to-back in one NEFF - used only for timing
(the slope of wall time in K isolates per-body device time from RPC
dispatch costs). The graded kernel() path uses rep=1.
"""
import numpy as np

import concourse.bacc as bacc
import concourse.bass as bass
import concourse.mybir as mybir
import concourse.tile as tile
from concourse import bass_utils

f32 = mybir.dt.float32
f16 = mybir.dt.float16
u32 = mybir.dt.uint32
AluOp = mybir.AluOpType
AFT = mybir.ActivationFunctionType
AX = mybir.AxisListType

N, D, A = 16384, 1024, 128
NT = N // 128           # 128 row-tiles
NG = NT // 4            # 32 groups of 4 tiles
SBG = 2                 # groups per DMA superblock (1024 rows)
NSB = NG // SBG         # 16 superblocks
KW = 132                # rhs width: 128 h + 3 logit diffs + 1 bag score
NEG = -1.0e30


def build_kernel(stage=99, rep=1, with_b1=False):
    nc = bacc.Bacc("TRN2", target_bir_lowering=False, debug=False, num_devices=8)
    # Xt is stored in stream order: [sb, p, c, nn] so each superblock DMA
    # reads one contiguous 16 KB block per partition (sequential HBM access).
    Xt = nc.dram_tensor("Xt", [NSB, 128, 8, SBG * 512], f16,
                        kind="ExternalInput").ap()
    Wa = nc.dram_tensor("Wa", [D, KW], f16, kind="ExternalInput").ap()
    w2r = nc.dram_tensor("w2r", [128, 8, 128], f32, kind="ExternalInput").ap()
    b1rep = (nc.dram_tensor("b1rep", [128, 2, 128], f32, kind="ExternalInput").ap()
             if with_b1 else None)
    maskg = nc.dram_tensor("maskg", [128, 128], f32, kind="ExternalInput").ap()
    cbr = nc.dram_tensor("cbr", [128, 4], f32, kind="ExternalInput").ap()
    out_vec = nc.dram_tensor("out_vec", [1, 8], f32, kind="ExternalOutput").ap()
    out_cnt = nc.dram_tensor("out_cnt", [2, 2], f32, kind="ExternalOutput").ap()

    with tile.TileContext(nc) as tc:
        consts = tc.alloc_tile_pool(name="consts", bufs=1)
        Wasb = consts.tile([128, 8, KW], f16)
        nc.sync.dma_start(Wasb[:], Wa.rearrange("(c p) k -> p c k", p=128))
        w2sb = consts.tile([128, 8, 128], f32)
        nc.sync.dma_start(w2sb[:], w2r[:])
        if with_b1:
            b1sb = consts.tile([128, 2, 128], f32)
            nc.sync.dma_start(b1sb[:], b1rep[:])
        masksb = consts.tile([128, 128], f32)
        nc.sync.dma_start(masksb[:], maskg[:])
        mask01 = consts.tile([128, 128], f32)
        nc.vector.tensor_scalar(mask01[:], masksb[:], 0.0, None, op0=AluOp.is_gt)
        ones1 = consts.tile([1, 128], f32)
        nc.vector.memset(ones1[:], 1.0)
        cbsb = consts.tile([128, 4], f32)
        nc.sync.dma_start(cbsb[:], cbr[:])
        onesc = consts.tile([128, 4], f32)
        nc.vector.memset(onesc[:], 1.0)

        # persistent grids: [p, t] = row n = 128*t + p
        u_grid = consts.tile([128, 128], f32)     # exp(f)
        w_grid = consts.tile([128, 128], f32)     # u * mask01
        Lgrid = consts.tile([128, 128, 4], f32)   # logit diffs 0..2, col 3 = c_n

        def emit_body():
            xp = tc.alloc_tile_pool(name="xp", bufs=2)
            thp = tc.alloc_tile_pool(name="thp", bufs=2)
            ps_h = tc.alloc_tile_pool(name="ps_h", bufs=3, space="PSUM")

            for sb in range(NSB):
                xsb = xp.tile([128, 8, SBG * 512], f16, name=f"x{sb}", tag="x",
                              bufs=4)
                nc.sync.dma_start(xsb[:, 0:4], Xt[sb, :, 0:4])
                nc.sync.dma_start(xsb[:, 4:8], Xt[sb, :, 4:8])
                if stage == 0:
                    # DMA-only ablation: touch the tile so it isn't dead
                    probe = thp.tile([128, 1], f16, name=f"pr{sb}", tag="pr")
                    nc.vector.tensor_copy(probe[:], xsb[:, 0, 0:1])
                    continue
                phs = []
                for gi in range(SBG):
                    g = sb * SBG + gi
                    for h2 in range(2):
                        ph = ps_h.tile([128, 2, KW], f32, name=f"ph{g}_{h2}",
                                       tag=f"ph{h2}", bufs=4)
                        phs.append(ph)
                        for tt in range(2):
                            off = gi * 512 + (2 * h2 + tt) * 128
                            for c in range(8):
                                nc.tensor.matmul(ph[:, tt, :],
                                                 xsb[:, c, off:off + 128],
                                                 Wasb[:, c, :],
                                                 start=(c == 0), stop=(c == 7))
                        if with_b1:
                            nc.vector.tensor_tensor(
                                ph[:, :, 0:128], ph[:, :, 0:128],
                                b1sb[:], op=AluOp.add)
                if stage == 1:
                    probe = thp.tile([128, 1], f32, name=f"pg{sb}", tag="pr1")
                    nc.vector.tensor_copy(probe[:], phs[0][:, 0, 0:1])
                    continue
                # tanh straight from PSUM, then f = sum_a th * w2
                th = thp.tile([128, 8, 128], f32, name=f"th{sb}", tag="th")
                for q in range(4):
                    nc.scalar.activation(th[:, 2 * q:2 * q + 2, :],
                                         phs[q][:, :, 0:128], AFT.Tanh,
                                         bias=0.0, scale=1.0)
                scr = thp.tile([128, 8, 128], f32, name=f"sc{sb}", tag="sc")
                nc.vector.tensor_tensor(scr[:], th[:], w2sb[:], op=AluOp.mult)
                fcol = thp.tile([128, 8], f32, name=f"f{sb}", tag="f")
                nc.vector.tensor_reduce(
                    fcol[:].rearrange("p (f o) -> p f o", o=1),
                    scr[:], axis=AX.X, op=AluOp.add)
                nc.scalar.activation(u_grid[:, 8 * sb:8 * sb + 8], fcol[:],
                                     AFT.Exp, bias=0.0, scale=1.0)
                nc.vector.tensor_tensor(w_grid[:, 8 * sb:8 * sb + 8],
                                        u_grid[:, 8 * sb:8 * sb + 8],
                                        mask01[:, 8 * sb:8 * sb + 8],
                                        op=AluOp.mult)
                # logit diffs (+cb) and bag scores into Lgrid
                for q in range(4):
                    dst = Lgrid[:, 8 * sb + 2 * q:8 * sb + 2 * q + 2, :]
                    if q % 2 == 0:
                        nc.vector.tensor_copy(dst, phs[q][:, :, 128:132])
                    else:
                        nc.scalar.copy(dst, phs[q][:, :, 128:132])

            ps_h.release()

            if stage <= 2:
                tailp = tc.alloc_tile_pool(name="tailp", bufs=1)
                outt = tailp.tile([1, 8], f32)
                nc.vector.memset(outt[:], 0.0)
                nc.sync.dma_start(out_vec[:], outt[:])
                cnts = tailp.tile([2, 2], f32)
                nc.vector.memset(cnts[:], 0.0)
                nc.sync.dma_start(out_cnt[:], cnts[:])
                tailp.release()
                thp.release()
                xp.release()
                return

            # ---------- tail ----------
            tailp = tc.alloc_tile_pool(name="tailp", bufs=1)
            ps_t = tc.alloc_tile_pool(name="ps_t", bufs=1, space="PSUM")

            # L = sum(w_grid); bag dot = sum(w * c)
            S4 = tailp.tile([128, 4], f32)
            nc.vector.tensor_reduce(S4[:, 0:1], w_grid[:], axis=AX.X, op=AluOp.add)
            pL = ps_t.tile([1, 4], f32)
            nc.tensor.matmul(pL[:], S4[:, 0:1], onesc[:], start=True, stop=True)
            recipL = tailp.tile([1, 1], f32)
            nc.vector.reciprocal(recipL[:], pL[:, 0:1])
            cw = tailp.tile([128, 128], f32)
            nc.vector.tensor_tensor(cw[:], w_grid[:], Lgrid[:, :, 3], op=AluOp.mult)

            # softplus grids depend only on Lgrid: run on ACT while the DVE
            # does the rank rounds below
            sps = []
            for k in range(3):
                ee = tailp.tile([128, 128], f32, name=f"ee{k}")
                nc.scalar.activation(ee[:], Lgrid[:, :, k], AFT.Exp,
                                     bias=cbsb[:, k:k + 1], scale=1.0)
                sp = tailp.tile([128, 128], f32, name=f"sp{k}")
                nc.scalar.activation(sp[:], ee[:], AFT.Ln, bias=1.0, scale=1.0)
                sps.append(sp)

            # candidates: top-8 per partition of u (and of -u)
            v8 = tailp.tile([128, 8], f32)
            nc.vector.max(v8[:], u_grid[:])
            uneg = tailp.tile([128, 128], f32)
            nc.vector.tensor_scalar(uneg[:], u_grid[:], -1.0, None, op0=AluOp.mult)
            v8b = tailp.tile([128, 8], f32)
            nc.vector.max(v8b[:], uneg[:])

            # consolidate candidate values to [2, 1024] rows (p-major: col = 8p+c)
            cand2 = tailp.tile([2, 1024], f32)
            nc.sync.dma_start(cand2[0:1, :], v8[:])
            nc.sync.dma_start(cand2[1:2, :], v8b[:])
            candB0 = tailp.tile([1, 1024], f32)
            nc.sync.dma_start(candB0[:], v8b[:])

            # threshold: 8 rounds of max8 + match_replace -> 64th; one more
            # max8 -> 65th; thr = midpoint
            work = tailp.tile([2, 1024], f32)
            nc.vector.tensor_copy(work[:], cand2[:])
            m8 = tailp.tile([2, 8], f32)
            v64 = tailp.tile([2, 1], f32)
            for r in range(8):
                nc.vector.max(m8[:], work[:])
                if r == 7:
                    nc.vector.tensor_copy(v64[:], m8[:, 7:8])
                nc.vector.match_replace(work[:], m8[:], work[:], NEG)
            m8b = tailp.tile([2, 8], f32)
            nc.vector.max(m8b[:], work[:])
            thr2 = tailp.tile([2, 1], f32)
            nc.vector.tensor_scalar(thr2[:], v64[:], m8b[:, 0:1], 0.5,
                                    op0=AluOp.add, op1=AluOp.mult)

            # candidate-space selections -> counts + 8th-slot guard (out_cnt)
            selTB = tailp.tile([2, 1024], f32)
            nc.vector.tensor_scalar(selTB[:], cand2[:], thr2[:, :1], None,
                                    op0=AluOp.is_gt)
            cnt2 = tailp.tile([2, 2], f32)
            nc.vector.tensor_reduce(cnt2[:, 0:1], selTB[:], axis=AX.X, op=AluOp.add)
            nc.vector.tensor_reduce(
                cnt2[:, 1:2].rearrange("q (a o) -> q a o", a=1),
                selTB[:].rearrange("q (p j) -> q p j", p=128)[:, :, 7:8],
                axis=AX.XY, op=AluOp.add)
            nc.sync.dma_start(out_cnt[:], cnt2[:])

            # broadcast thresholds to all partitions: thrps = ones1^T @ [thrT thrB]
            thrrow = tailp.tile([1, 2], f32)
            nc.sync.dma_start(thrrow[:], thr2[:])
            thrps = ps_t.tile([128, 2], f32)
            nc.tensor.matmul(thrps[:], ones1[:], thrrow[:], start=True, stop=True)
            thrsb = tailp.tile([128, 2], f32)
            nc.vector.tensor_copy(thrsb[:], thrps[:])

            # grid-space selections
            selgT = tailp.tile([128, 128], f32)
            nc.vector.tensor_scalar(selgT[:], u_grid[:], thrsb[:, 0:1], None,
                                    op0=AluOp.is_gt)
            selgB = tailp.tile([128, 128], f32)
            nc.vector.tensor_scalar(selgB[:], uneg[:], thrsb[:, 1:2], None,
                                    op0=AluOp.is_gt)

            # masked sums over the precomputed softplus grids
            # slot 1: top sel, diff col 0; slot 2: bottom sel, col 1;
            # slot 3: top sel, col 2
            for slot, (k, selg) in enumerate(
                    [(0, selgT), (1, selgB), (2, selgT)], start=1):
                ws = tailp.tile([128, 128], f32, name=f"ws{k}{slot}")
                nc.vector.tensor_tensor(ws[:], sps[k][:], selg[:], op=AluOp.mult)
                nc.vector.tensor_reduce(S4[:, slot:slot + 1], ws[:], axis=AX.X,
                                        op=AluOp.add)
            # overwrite S4 col 0 with the bag dot now that cw is ready
            nc.vector.tensor_reduce(S4[:, 0:1], cw[:], axis=AX.X, op=AluOp.add)

            # partition sums: pS[m, j] = sum_p S4[p, m]; slot 0 scaled by 1/L
            pS = ps_t.tile([4, 4], f32)
            nc.tensor.matmul(pS[:], S4[:], onesc[:], start=True, stop=True)
            psb = tailp.tile([4, 1], f32)
            nc.scalar.copy(psb[:], pS[:, 0:1])
            nc.vector.tensor_scalar(psb[0:1, :], psb[0:1, :], recipL[:, :1],
                                    None, op0=AluOp.mult)
            nc.sync.dma_start(out_vec[:, 0:4], psb[:])

            ps_t.release()
            tailp.release()
            thp.release()
            xp.release()

        if rep == 1:
            emit_body()
        else:
            with tc.For_i(0, rep, 1, hint_engines=tuple(mybir.ALL_ENGINES)):
                emit_body()

        consts.release()

    nc.compile()
    return nc


_NC_CACHE = {}


def _get_nc(with_b1=False):
    global _NC_CACHE
    if _NC_CACHE is None:
        _NC_CACHE = {}
    if with_b1 not in _NC_CACHE:
        import os
        _NC_CACHE[with_b1] = build_kernel(
            int(os.environ.get("KSTAGE", "99")),
            rep=int(os.environ.get("KREP", "1")), with_b1=with_b1)
    return _NC_CACHE[with_b1]


def make_in_maps(X, mask, labels, W1, b1, w2, b2, Wc, bc, Wi, bi):
    
    X = np.asarray(X, dtype=np.float32)
    mask = np.asarray(mask, dtype=np.float32)
    labels = np.asarray(labels).astype(np.int64)
    W1 = np.asarray(W1, dtype=np.float32)
    b1v = np.asarray(b1, dtype=np.float32).reshape(1, 1, A)
    w2v = np.asarray(w2, dtype=np.float32).reshape(1, 1, A)
    Wc = np.asarray(Wc, dtype=np.float32).reshape(D, 1)
    Wi = np.asarray(Wi, dtype=np.float32)
    bi = np.asarray(bi, dtype=np.float32)
    w2r = np.ascontiguousarray(np.broadcast_to(w2v, (128, 8, A)))
    b1f = np.asarray(b1, dtype=np.float32).reshape(1, A)
    with_b1 = bool(np.any(b1f))
    b1rep = np.ascontiguousarray(
        np.broadcast_to(b1f.reshape(1, 1, A), (128, 2, A)))
    in_maps = []
    for b in range(8):
        lab = int(labels[b])
        Win, Wout = Wi[lab], Wi[1 - lab]
        Wd3 = np.stack([Win[:, 0] - Win[:, 1],
                        Win[:, 1] - Win[:, 0],
                        Wout[:, 1] - Wout[:, 0]], axis=1)  # [1024, 3]
        Wa = np.concatenate([W1, Wd3, Wc], axis=1).astype(np.float16)  # [1024, 132]
        bin_, bout = bi[lab], bi[1 - lab]
        cb = np.array([1.0 + bin_[0] - bin_[1],
                       1.0 + bin_[1] - bin_[0],
                       1.0 + bout[1] - bout[0], 0.0], dtype=np.float32)
        cbrep = np.ascontiguousarray(np.broadcast_to(cb.reshape(1, 4), (128, 4)))
        maskgrid = np.ascontiguousarray(mask[b].reshape(128, 128).T)
        Xtb = X[b].astype(np.float16).T  # [1024(d), 16384(n)] view
        # stream-order layout [sb, p, c, nn]: d = 128*c + p, n = SBG*512*sb + nn
        Xts = np.ascontiguousarray(
            Xtb.reshape(8, 128, 32 // SBG, SBG * 512).transpose(2, 1, 0, 3))
        in_maps.append({
            "Xt": Xts,
            "Wa": np.ascontiguousarray(Wa),
            "w2r": w2r,
            "maskg": maskgrid,
            "cbr": cbrep,
        })
        if with_b1:
            in_maps[-1]["b1rep"] = b1rep
    return in_maps


def assemble(results, labels, bc):
    labels = np.asarray(labels).astype(np.float64)
    bag_pred = np.zeros(8, dtype=np.float64)
    inst = 0.0
    for b in range(8):
        ov = results[b]["out_vec"][0].astype(np.float64)
        bag_pred[b] = ov[0] + float(np.asarray(bc).reshape(-1)[0])
        inst += (ov[1] + ov[2]) / 128.0 + ov[3] / 64.0
    crit = np.mean(np.logaddexp(0.0, bag_pred) - bag_pred * labels)
    out = np.concatenate([bag_pred, [crit], [inst]]).astype(np.float32)
    return out


def kernel(X, mask, labels, W1, b1, w2, b2, Wc, bc, Wi, bi):
    nc = _get_nc(with_b1=bool(np.any(np.asarray(b1))))
    in_maps = make_in_maps(X, mask, labels, W1, b1, w2, b2, Wc, bc, Wi, bi)
    res = bass_utils.run_bass_kernel_spmd(nc, in_maps, core_ids=list(range(8)))
    return assemble(res.results, labels, bc)
